# revision 18
# baseline (speedup 1.0000x reference)
"""Self-contained Trainium2 Bass kernel for causal self-MQA.

Reference semantics (S=2048, B=2, D=2048, H=16 heads, dqk=dv=128):
  q  = hs @ q_w.T + q_b ;  kv = hs @ kv_w.T + kv_b  (single shared KV head)
  scores = causal-masked q.k / sqrt(dqk);  attn = softmax;  out = (attn.v) @ o_w.T + o_b

Sharding (8 cores, no collectives): data-parallel over batch (2 groups of 4
cores) x sequence-parallel over interleaved query tiles.  Core c handles batch
c//4 and query tiles {r, r+4, r+8, r+12} (r = c%4, tiles of 128 rows).  The
SPMD program is identical on every core: q-slot j processes k-blocks 0..4j+3
and the r-dependent causal boundary is encoded in host-provided mask tiles
(ones / lower-triangular / zeros) multiplied onto the first k-block column of
each k-block's window.

Engine plan (v2): PE does all matmuls (projections, scores, PV, denominator
reduction + broadcast); ACT does one wide exp per (head-pair, k-block); Pool
(gpsimd) does mask-mult, denominator accumulation and the final normalize;
DVE does PSUM evictions and the approx reciprocal.  Q-projection is folded
into the attention loop (per head-pair) so the PE stream never drains.
"""

import sys

sys.path.insert(0, "/opt/trn_rl_repo")

import numpy as np
import ml_dtypes
from contextlib import ExitStack

import concourse.bass as bass
import concourse.mybir as mybir
import concourse.tile as tile
from concourse.tile import add_dep_helper
from concourse import bacc
from concourse.bass_utils import run_bass_kernel_spmd

F32 = mybir.dt.float32
F32R = mybir.dt.float32r
BF16 = mybir.dt.bfloat16
FP16 = mybir.dt.float16

# per-stage matmul dtypes (knobs)
Q_DT = BF16     # q-projection
KV_DT = FP16    # kv-projection
A_DT = FP16     # scores / PV matmuls
O_DT = FP16     # o-projection

_NP_OF = {BF16: ml_dtypes.bfloat16, FP16: np.float16, F32: np.float32}

SEQ, BATCH, DMODEL, NH, DQK = 2048, 2, 2048, 16, 128
NCORE = 8
NEG = -30000.0


def _build(seq, dmodel, nh):
    """Build + compile the SPMD program for one core's shard."""
    T = seq // 128            # k-blocks
    NSLOT = T // 4            # q-tiles per core
    NQ = NSLOT * 128          # query rows per core
    IC = dmodel // 128        # contraction chunks for projections
    NS = seq // 512           # 512-wide seq tiles
    ND = dmodel // 512        # 512-wide d_model tiles
    HG = 2                    # heads per group (pair)
    NHG = nh // HG
    SCALE = 1.0 / float(np.sqrt(DQK))

    nc = bacc.Bacc("TRN2", target_bir_lowering=False, debug=False,
                   num_devices=NCORE)

    hsT = nc.dram_tensor("hsT", [dmodel, seq], KV_DT, kind="ExternalInput")
    hsq = nc.dram_tensor("hsq", [dmodel, NQ], Q_DT, kind="ExternalInput")
    qwT = nc.dram_tensor("qwT", [dmodel, nh * 128], Q_DT, kind="ExternalInput")
    kvwT = nc.dram_tensor("kvwT", [dmodel, 256], KV_DT, kind="ExternalInput")
    owT = nc.dram_tensor("owT", [nh * 128, dmodel], O_DT, kind="ExternalInput")
    qb = nc.dram_tensor("qb", [128, nh], F32, kind="ExternalInput")
    kvb = nc.dram_tensor("kvb", [128, 2], F32, kind="ExternalInput")
    obias = nc.dram_tensor("obias", [1, dmodel], O_DT, kind="ExternalInput")
    logmask = nc.dram_tensor("logmask", [128, T], F32, kind="ExternalInput")
    bmask = nc.dram_tensor("bmask", [128, 4 * 128], A_DT, kind="ExternalInput")
    ident_in = nc.dram_tensor("ident", [128, 128], A_DT, kind="ExternalInput")
    ones_in = nc.dram_tensor("ones_in", [128, 128], F32, kind="ExternalInput")
    ones_o_in = nc.dram_tensor("ones_o", [1, 128], O_DT, kind="ExternalInput")
    out = nc.dram_tensor("out", [NQ, dmodel], O_DT, kind="ExternalOutput")

    def j0_of(kb):
        # first q-slot whose window 0..4j+3 contains kb
        return max(0, -(-(kb - 3) // 4))

    with tile.TileContext(nc) as tc, ExitStack() as ctx:
        pers = ctx.enter_context(tc.tile_pool(name="pers", bufs=1))
        ow_cm = tc.tile_pool(name="owp", bufs=1)
        owp = ow_cm.__enter__()
        aio_cm = tc.tile_pool(name="attn_io", bufs=1)
        aio = aio_cm.__enter__()
        kT = aio.tile([128, seq], A_DT)
        v = aio.tile([128, T, 128], A_DT)           # v natural, chunked by k-block
        qT = aio.tile([128, nh, NQ], A_DT)
        hsq_sb = aio.tile([128, IC, NQ], Q_DT)
        attnT = pers.tile([128, nh, NQ], O_DT)      # normalized attn out (hd, q)
        qb_sb = pers.tile([128, nh], F32)
        kvb_sb = pers.tile([128, 2], F32)
        lm_sb = pers.tile([128, T], F32)
        bm_sb = pers.tile([128, 4, 128], A_DT)   # additive -30000 causal mask
        ob_sb = pers.tile([1, dmodel], O_DT)
        ident = pers.tile([128, 128], A_DT)
        ones_f32 = pers.tile([128, 128], F32)
        ones_a = pers.tile([128, 128], A_DT)
        ones_row_o = pers.tile([1, 128], O_DT)

        nc.scalar.dma_start(out=qb_sb[:], in_=qb.ap())
        nc.scalar.dma_start(out=kvb_sb[:], in_=kvb.ap())
        nc.scalar.dma_start(out=lm_sb[:], in_=logmask.ap())
        nc.scalar.dma_start(out=bm_sb[:],
                            in_=bmask.ap().rearrange("p (m q) -> p m q", m=4))
        nc.scalar.dma_start(out=ob_sb[:], in_=obias.ap())
        nc.scalar.dma_start(out=ident[:], in_=ident_in.ap())
        nc.scalar.dma_start(out=ones_f32[:], in_=ones_in.ap())
        nc.scalar.dma_start(out=ones_row_o[:], in_=ones_o_in.ap())
        nc.vector.tensor_copy(ones_a[:], ones_f32[:])

        # ---------------- phase KV: kT = kv_w[:128] @ hsT, vT -> v ----------------
        vT = None
        with tc.tile_pool(name="kvw", bufs=1) as kvwp, \
             tc.tile_pool(name="vtp", bufs=1) as vtp:
            kvw_sb = kvwp.tile([128, IC, 256], KV_DT)
            kvw_ap = kvwT.ap().rearrange("(i p) c -> p i c", p=128)
            nc.sync.dma_start(out=kvw_sb[:, :, 0:128], in_=kvw_ap[:, :, 0:128])
            nc.scalar.dma_start(out=kvw_sb[:, :, 128:256], in_=kvw_ap[:, :, 128:256])
            vT = vtp.tile([128, seq], A_DT)
            with tc.tile_pool(name="hstream", bufs=6) as hsp, \
                 tc.tile_pool(name="pskv", bufs=1, space="PSUM") as pskv:
                psk = [pskv.tile([128, 512], F32, tag=f"psk{s}", name=f"psk{s}") for s in range(NS)]
                psv = [pskv.tile([128, 512], F32, tag=f"psv{s}", name=f"psv{s}") for s in range(NS)]
                h_dmas = []
                for i in range(IC):
                    h = hsp.tile([128, seq], KV_DT, tag="hst", name="hst")
                    eng = nc.sync if i % 2 == 0 else nc.scalar
                    h_dmas.append(
                        eng.dma_start(out=h[:], in_=hsT.ap()[i * 128:(i + 1) * 128, :]))
                    for s in range(NS):
                        nc.tensor.matmul(psk[s][:], kvw_sb[:, i, 0:128],
                                         h[:, s * 512:(s + 1) * 512],
                                         start=(i == 0), stop=(i == IC - 1))
                        nc.tensor.matmul(psv[s][:], kvw_sb[:, i, 128:256],
                                         h[:, s * 512:(s + 1) * 512],
                                         start=(i == 0), stop=(i == IC - 1))
                # hsq prefetch rides the gpsimd (SWDGE) queue, gated behind
                # the kv stream so it doesn't steal its HBM bandwidth
                hsq_dma = nc.gpsimd.dma_start(
                    out=hsq_sb[:],
                    in_=hsq.ap().rearrange("(i p) q -> p i q", p=128))
                add_dep_helper(hsq_dma.ins, h_dmas[13].ins,
                               reason="pace hsq behind the kv stream")
                for s in range(NS):
                    nc.scalar.activation(kT[:, s * 512:(s + 1) * 512], psk[s][:],
                                         mybir.ActivationFunctionType.Identity,
                                         bias=kvb_sb[:, 0:1])
                    nc.vector.tensor_scalar_add(vT[:, s * 512:(s + 1) * 512], psv[s][:],
                                                kvb_sb[:, 1:2])
            with tc.tile_pool(name="pst", bufs=4, space="PSUM") as pst:
                for t in range(T):
                    pt = pst.tile([128, 128], A_DT, tag="pt")
                    nc.tensor.transpose(pt[:], vT[:, t * 128:(t + 1) * 128], ident[:])
                    if t % 2 == 0:
                        nc.vector.tensor_copy(v[:, t, :], pt[:])
                    else:
                        nc.scalar.activation(v[:, t, :], pt[:],
                                             mybir.ActivationFunctionType.Copy)

        # ---------------- phase A: q-proj + scores -> exp -> PV, per head-pair ----
        ow_sb = owp.tile([128, nh, dmodel], O_DT)
        qw_cm = tc.tile_pool(name="qwp", bufs=2)
        qwp = qw_cm.__enter__()
        dn_cm = tc.tile_pool(name="dnp", bufs=2)
        dnp = dn_cm.__enter__()
        rs_cm = tc.tile_pool(name="rsp", bufs=2)
        rsp = rs_cm.__enter__()
        pexp_cm = tc.tile_pool(name="pexp", bufs=6)
        pexp = pexp_cm.__enter__()
        psS_cm = tc.tile_pool(name="psS", bufs=3, space="PSUM")
        psSp = psS_cm.__enter__()
        psU_cm = tc.tile_pool(name="psU", bufs=1, space="PSUM")
        psUp = psU_cm.__enter__()

        kv_gate = h_dmas  # kv-stream DMAs; pace phase-A prefetches behind them

        tail = {}   # deferred tail work of the previous head-pair

        def run_tail():
            if not tail:
                return
            heads, dn, psu = tail["heads"], tail["dn"], tail["psu"]
            # evict unnormalized PV accumulators
            for hh, hd in enumerate(heads):
                nc.vector.tensor_copy(attnT[:, hd, :], psu[hh][:])
            # denominator: partition-reduce dn via M=1 ones matmuls.  pd reuses
            # the freshly-evicted psu0 bank, the broadcast pr reuses psu1, so
            # the scores ("ss") rotation never couples to this tail work.
            pd_of = {0: psUp.tile([128, NQ], F32, tag="psu0", name="pd0"),
                     1: psUp.tile([128, NQ], F32, tag="psu1", name="pd1")}
            for hh in range(HG):
                nc.tensor.matmul(pd_of[hh][0:1, :], ones_a[:, 0:1],
                                 dn[:, hh, :], start=True, stop=True,
                                 skip_group_check=True)
            rs = rsp.tile([128, HG, NQ], F32, tag="rs", name="rs")
            rsh = rsp.tile([128, HG, NQ], A_DT, tag="rsh", name="rsh")
            for hh in range(HG):
                nc.vector.reciprocal_approx_fast(
                    rs[0:1, hh, :], pd_of[hh][0:1, :])
                with nc.allow_low_precision(reason="1/denom in fp16 is plenty"):
                    nc.vector.tensor_copy(rsh[0:1, hh, :], rs[0:1, hh, :])
            pbr = {0: psUp.tile([128, NQ], F32, tag="psu0", name="pbr0"),
                   1: psUp.tile([128, NQ], F32, tag="psu1", name="pbr1")}
            for hh, hd in enumerate(heads):
                nc.tensor.matmul(pbr[hh][:], ones_a[0:1, 0:128],
                                 rsh[0:1, hh, :],
                                 start=True, stop=True, skip_group_check=True)
                nc.vector.tensor_mul(attnT[:, hd, :], attnT[:, hd, :], pbr[hh][:])
            tail.clear()

        for hg in range(NHG):
            heads = [hg * HG + i for i in range(HG)]
            # stream this pair's weights: qw chunk (sync q), ow rows (scalar q)
            qwg = qwp.tile([128, IC, HG * 128], Q_DT, tag="qwg", name="qwg")
            qwg_dma = nc.gpsimd.dma_start(
                out=qwg[:],
                in_=qwT.ap()[:, hg * HG * 128:(hg + 1) * HG * 128]
                .rearrange("(i p) o -> p i o", p=128))
            if hg < 2:
                add_dep_helper(qwg_dma.ins, kv_gate[11 + 2 * hg].ins,
                               reason="pace qw prefetch behind the kv stream")
            for hd in heads:
                nc.sync.dma_start(out=ow_sb[:, hd, :],
                                  in_=owT.ap()[hd * 128:(hd + 1) * 128, :])
            # q-projection for the pair (uses the "ss" PSUM region)
            psq = psSp.tile([128, HG, 512], F32, tag="ss", name="psq")
            for hh in range(HG):
                for i in range(IC):
                    nc.tensor.matmul(psq[:, hh, :], qwg[:, i, hh * 128:(hh + 1) * 128],
                                     hsq_sb[:, i, :],
                                     start=(i == 0), stop=(i == IC - 1),
                                     skip_group_check=True)
            for hh, hd in enumerate(heads):
                nc.vector.tensor_scalar_add(qT[:, hd, :], psq[:, hh, :],
                                            qb_sb[:, hd:hd + 1])
            # deferred tail of the previous pair (keeps PE fed during our exps)
            run_tail()

            dn = dnp.tile([128, HG, NQ], A_DT, tag="dn", name="dn")
            nc.gpsimd.memset(dn[:], 0.0)
            psu = [psUp.tile([128, NQ], F32, tag=f"psu{hh}", name=f"psu{hh}")
                   for hh in range(HG)]
            pipe = []   # software-pipelined PV: run PV(kb-2) after scores(kb)
            for kb in range(T):
                j0 = j0_of(kb)
                ncols = (NSLOT - j0) * 128
                m = kb % 4
                ss = psSp.tile([128, HG, 512], F32, tag="ss", name="ss")
                for hh, hd in enumerate(heads):
                    nc.tensor.matmul(ss[:, hh, :ncols], kT[:, kb * 128:(kb + 1) * 128],
                                     qT[:, hd, j0 * 128:NQ], start=True, stop=False,
                                     skip_group_check=True)
                for hh in range(HG):
                    # boundary-block causal mask: ss[:, hh, 0:128] += -30000*(~allowed)
                    nc.tensor.matmul(ss[:, hh, 0:128], ident[:], bm_sb[:, m],
                                     start=False, stop=True, skip_group_check=True)
                p = pexp.tile([128, HG, 512], A_DT, tag="p", name="p")
                nc.scalar.activation(p[:, :, :ncols], ss[:, :, :ncols],
                                     mybir.ActivationFunctionType.Exp,
                                     bias=lm_sb[:, kb:kb + 1], scale=SCALE)
                nc.vector.tensor_add(dn[:, :, j0 * 128:NQ], dn[:, :, j0 * 128:NQ],
                                     p[:, :, :ncols])
                pipe.append((kb, j0, ncols, p))
                if len(pipe) > 2:
                    pkb, pj0, pnc, pp = pipe.pop(0)
                    for hh in range(HG):
                        nc.tensor.matmul(psu[hh][:, pj0 * 128:NQ], v[:, pkb, :],
                                         pp[:, hh, :pnc],
                                         start=(pkb == 0), stop=False,
                                         skip_group_check=True)
            for pkb, pj0, pnc, pp in pipe:
                for hh in range(HG):
                    nc.tensor.matmul(psu[hh][:, pj0 * 128:NQ], v[:, pkb, :],
                                     pp[:, hh, :pnc],
                                     start=(pkb == 0), stop=(pkb == T - 1),
                                     skip_group_check=True)
            tail.update({"heads": heads, "dn": dn, "psu": psu})
        run_tail()

        psU_cm.__exit__(None, None, None)
        psS_cm.__exit__(None, None, None)
        pexp_cm.__exit__(None, None, None)
        rs_cm.__exit__(None, None, None)
        dn_cm.__exit__(None, None, None)
        qw_cm.__exit__(None, None, None)
        aio_cm.__exit__(None, None, None)

        # ---------------- phase O: out = attnT.T @ owT + o_b ----------------
        with tc.tile_pool(name="psO", bufs=2, space="PSUM") as psOp, \
             tc.tile_pool(name="ost", bufs=2) as ostp:
            for sp in range(NSLOT):
                pso = {dt: psOp.tile([128, 512], F32, tag=f"pso{dt}",
                                     name=f"pso{dt}")
                       for dt in range(ND)}
                for ih in range(nh):
                    for dt in range(ND):
                        nc.tensor.matmul(pso[dt][:],
                                         attnT[:, ih, sp * 128:(sp + 1) * 128],
                                         ow_sb[:, ih, dt * 512:(dt + 1) * 512],
                                         start=(ih == 0), stop=False,
                                         skip_group_check=True)
                for dt in range(ND):
                    nc.tensor.matmul(pso[dt][:], ones_row_o[:],
                                     ob_sb[:, dt * 512:(dt + 1) * 512],
                                     start=False, stop=True, skip_group_check=True)
                for dt in range(ND):
                    og = ostp.tile([128, 512], O_DT, tag=f"og{dt % 2}", name="og")
                    nc.vector.tensor_copy(og[:], pso[dt][:])
                    eng = nc.sync if dt % 2 == 0 else nc.scalar
                    eng.dma_start(
                        out=out.ap()[sp * 128:(sp + 1) * 128,
                                     dt * 512:(dt + 1) * 512],
                        in_=og[:])
        ow_cm.__exit__(None, None, None)

    nc.compile()
    return nc


def make_in_maps(hidden_states, sequence_mask, q_w, q_b, kv_w, kv_b, o_w, o_b,
                 seq, dmodel, nh):
    """Host-side shard prep -> list of 8 per-core input dicts."""
    T = seq // 128
    NSLOT = T // 4
    npq, npkv, npa, npo = _NP_OF[Q_DT], _NP_OF[KV_DT], _NP_OF[A_DT], _NP_OF[O_DT]
    f32 = np.float32

    qwT = np.ascontiguousarray(q_w.astype(f32).T).astype(npq)
    kvwT = np.ascontiguousarray(kv_w.astype(f32).T).astype(npkv)
    owT = np.ascontiguousarray(o_w.astype(f32).T).astype(npo)
    qb2 = np.ascontiguousarray(q_b.astype(f32).reshape(nh, 128).T)
    kvb2 = np.ascontiguousarray(kv_b.astype(f32).reshape(2, 128).T)
    ob2 = o_b.astype(f32).reshape(1, dmodel).astype(npo)
    ident = np.eye(128, dtype=npa)
    ones128 = np.ones((128, 128), dtype=f32)
    ones_o = np.ones((1, 128), dtype=npo)
    tri = (np.arange(128)[None, :] >= np.arange(128)[:, None]).astype(f32)  # [k,q] q>=k

    in_maps = []
    for c in range(NCORE):
        b, r = divmod(c, 4)
        qtiles = [r + 4 * j for j in range(NSLOT)]
        hsT = np.ascontiguousarray(hidden_states[:, b, :].astype(f32).T)
        hsq = np.ascontiguousarray(
            np.concatenate([hsT[:, t * 128:(t + 1) * 128] for t in qtiles], axis=1))
        lm = np.where(sequence_mask[b].astype(np.int64) != 0, 0.0, NEG).astype(f32)
        lm = np.ascontiguousarray(lm.reshape(T, 128).T)
        bm = np.empty((128, 4 * 128), dtype=npa)
        for m in range(4):
            blk = (np.ones((128, 128), f32) if m < r else
                   (tri if m == r else np.zeros((128, 128), f32)))
            bm[:, m * 128:(m + 1) * 128] = (NEG * (1.0 - blk)).astype(npa)
        in_maps.append({
            "hsT": hsT.astype(npkv), "hsq": hsq.astype(npq), "qwT": qwT,
            "kvwT": kvwT, "owT": owT, "qb": qb2, "kvb": kvb2, "obias": ob2,
            "logmask": lm, "bmask": bm, "ident": ident, "ones_in": ones128,
            "ones_o": ones_o,
        })
    return in_maps


def assemble(results, seq, dmodel, nh):
    T = seq // 128
    NSLOT = T // 4
    full = np.empty((seq, BATCH, dmodel), np.float32)
    for c in range(NCORE):
        b, r = divmod(c, 4)
        o = results[c]["out"].astype(np.float32)
        for j in range(NSLOT):
            t = r + 4 * j
            full[t * 128:(t + 1) * 128, b, :] = o[j * 128:(j + 1) * 128, :]
    return full


_CACHE = {}


def kernel(hidden_states, sequence_mask, q_w, q_b, kv_w, kv_b, o_w, o_b):
    hidden_states = np.asarray(hidden_states)
    sequence_mask = np.asarray(sequence_mask)
    key = (SEQ, DMODEL, NH)
    if key not in _CACHE:
        _CACHE[key] = _build(SEQ, DMODEL, NH)
    nc = _CACHE[key]
    in_maps = make_in_maps(hidden_states, sequence_mask,
                           np.asarray(q_w), np.asarray(q_b), np.asarray(kv_w),
                           np.asarray(kv_b), np.asarray(o_w), np.asarray(o_b),
                           SEQ, DMODEL, NH)
    res = run_bass_kernel_spmd(nc, in_maps, core_ids=list(range(NCORE)))
    return assemble(res.results, SEQ, DMODEL, NH)


# revision 19
# speedup vs baseline: 1.0700x; 1.0700x over previous
"""Self-contained Trainium2 Bass kernel for causal self-MQA.

Reference semantics (S=2048, B=2, D=2048, H=16 heads, dqk=dv=128):
  q  = hs @ q_w.T + q_b ;  kv = hs @ kv_w.T + kv_b  (single shared KV head)
  scores = causal-masked q.k / sqrt(dqk);  attn = softmax;  out = (attn.v) @ o_w.T + o_b

Sharding (8 cores, no collectives): data-parallel over batch (2 groups of 4
cores) x sequence-parallel over interleaved query tiles.  Core c handles batch
c//4 and query tiles {r, r+4, r+8, r+12} (r = c%4, tiles of 128 rows).  The
SPMD program is identical on every core: q-slot j processes k-blocks 0..4j+3
and the r-dependent causal boundary is encoded in host-provided mask tiles
(ones / lower-triangular / zeros) multiplied onto the first k-block column of
each k-block's window.

Engine plan (v2): PE does all matmuls (projections, scores, PV, denominator
reduction + broadcast); ACT does one wide exp per (head-pair, k-block); Pool
(gpsimd) does mask-mult, denominator accumulation and the final normalize;
DVE does PSUM evictions and the approx reciprocal.  Q-projection is folded
into the attention loop (per head-pair) so the PE stream never drains.
"""

import sys

sys.path.insert(0, "/opt/trn_rl_repo")

import numpy as np
import ml_dtypes
from contextlib import ExitStack

import concourse.bass as bass
import concourse.mybir as mybir
import concourse.tile as tile
from concourse.tile import add_dep_helper
from concourse import bacc
from concourse.bass_utils import run_bass_kernel_spmd

F32 = mybir.dt.float32
F32R = mybir.dt.float32r
BF16 = mybir.dt.bfloat16
FP16 = mybir.dt.float16

# per-stage matmul dtypes (knobs)
Q_DT = BF16     # q-projection
KV_DT = FP16    # kv-projection
A_DT = FP16     # scores / PV matmuls
O_DT = FP16     # o-projection

_NP_OF = {BF16: ml_dtypes.bfloat16, FP16: np.float16, F32: np.float32}

SEQ, BATCH, DMODEL, NH, DQK = 2048, 2, 2048, 16, 128
NCORE = 8
NEG = -30000.0


def _build(seq, dmodel, nh):
    """Build + compile the SPMD program for one core's shard."""
    T = seq // 128            # k-blocks
    NSLOT = T // 4            # q-tiles per core
    NQ = NSLOT * 128          # query rows per core
    IC = dmodel // 128        # contraction chunks for projections
    NS = seq // 512           # 512-wide seq tiles
    ND = dmodel // 512        # 512-wide d_model tiles
    HG = 2                    # heads per group (pair)
    NHG = nh // HG
    SCALE = 1.0 / float(np.sqrt(DQK))

    nc = bacc.Bacc("TRN2", target_bir_lowering=False, debug=False,
                   num_devices=NCORE)

    hsT = nc.dram_tensor("hsT", [dmodel, seq], KV_DT, kind="ExternalInput")
    hsq = nc.dram_tensor("hsq", [dmodel, NQ], Q_DT, kind="ExternalInput")
    qwT = nc.dram_tensor("qwT", [dmodel, nh * 128], Q_DT, kind="ExternalInput")
    kvwT = nc.dram_tensor("kvwT", [dmodel, 256], KV_DT, kind="ExternalInput")
    owT = nc.dram_tensor("owT", [nh * 128, dmodel], O_DT, kind="ExternalInput")
    qb = nc.dram_tensor("qb", [128, nh], F32, kind="ExternalInput")
    kvb = nc.dram_tensor("kvb", [128, 2], F32, kind="ExternalInput")
    obias = nc.dram_tensor("obias", [1, dmodel], O_DT, kind="ExternalInput")
    logmask = nc.dram_tensor("logmask", [128, T], F32, kind="ExternalInput")
    bmask = nc.dram_tensor("bmask", [128, 4 * 128], A_DT, kind="ExternalInput")
    ident_in = nc.dram_tensor("ident", [128, 128], A_DT, kind="ExternalInput")
    ones_in = nc.dram_tensor("ones_in", [128, 128], F32, kind="ExternalInput")
    ones_o_in = nc.dram_tensor("ones_o", [1, 128], O_DT, kind="ExternalInput")
    out = nc.dram_tensor("out", [NQ, dmodel], O_DT, kind="ExternalOutput")

    def j0_of(kb):
        # first q-slot whose window 0..4j+3 contains kb
        return max(0, -(-(kb - 3) // 4))

    with tile.TileContext(nc) as tc, ExitStack() as ctx:
        pers = ctx.enter_context(tc.tile_pool(name="pers", bufs=1))
        ow_cm = tc.tile_pool(name="owp", bufs=1)
        owp = ow_cm.__enter__()
        aio_cm = tc.tile_pool(name="attn_io", bufs=1)
        aio = aio_cm.__enter__()
        kT = aio.tile([128, seq], A_DT)
        v = aio.tile([128, T, 128], A_DT)           # v natural, chunked by k-block
        qT = aio.tile([128, nh, NQ], A_DT)
        hsq_sb = aio.tile([128, IC, NQ], Q_DT)
        attnT = pers.tile([128, nh, NQ], O_DT)      # normalized attn out (hd, q)
        qb_sb = pers.tile([128, nh], F32)
        kvb_sb = pers.tile([128, 2], F32)
        lm_sb = pers.tile([128, T], F32)
        bm_sb = pers.tile([128, 4, 128], A_DT)   # additive -30000 causal mask
        ob_sb = pers.tile([1, dmodel], O_DT)
        ident = pers.tile([128, 128], A_DT)
        ones_f32 = pers.tile([128, 128], F32)
        ones_a = pers.tile([128, 128], A_DT)
        ones_row_o = pers.tile([1, 128], O_DT)

        nc.scalar.dma_start(out=qb_sb[:], in_=qb.ap())
        nc.scalar.dma_start(out=kvb_sb[:], in_=kvb.ap())
        nc.scalar.dma_start(out=lm_sb[:], in_=logmask.ap())
        nc.scalar.dma_start(out=bm_sb[:],
                            in_=bmask.ap().rearrange("p (m q) -> p m q", m=4))
        nc.scalar.dma_start(out=ob_sb[:], in_=obias.ap())
        nc.scalar.dma_start(out=ident[:], in_=ident_in.ap())
        nc.scalar.dma_start(out=ones_f32[:], in_=ones_in.ap())
        nc.scalar.dma_start(out=ones_row_o[:], in_=ones_o_in.ap())
        nc.vector.tensor_copy(ones_a[:], ones_f32[:])

        # ---------------- phase KV: kT = kv_w[:128] @ hsT, vT -> v ----------------
        vT = None
        with tc.tile_pool(name="kvw", bufs=1) as kvwp, \
             tc.tile_pool(name="vtp", bufs=1) as vtp:
            kvw_sb = kvwp.tile([128, IC, 256], KV_DT)
            kvw_ap = kvwT.ap().rearrange("(i p) c -> p i c", p=128)
            nc.sync.dma_start(out=kvw_sb[:, :, 0:128], in_=kvw_ap[:, :, 0:128])
            nc.scalar.dma_start(out=kvw_sb[:, :, 128:256], in_=kvw_ap[:, :, 128:256])
            vT = vtp.tile([128, seq], A_DT)
            with tc.tile_pool(name="hstream", bufs=8) as hsp, \
                 tc.tile_pool(name="pskv", bufs=1, space="PSUM") as pskv:
                psk = [pskv.tile([128, 512], F32, tag=f"psk{s}", name=f"psk{s}") for s in range(NS)]
                psv = [pskv.tile([128, 512], F32, tag=f"psv{s}", name=f"psv{s}") for s in range(NS)]
                h_dmas = []
                for i in range(IC):
                    h = hsp.tile([128, seq], KV_DT, tag="hst", name="hst")
                    hd0 = nc.sync.dma_start(
                        out=h[:, 0:seq // 2],
                        in_=hsT.ap()[i * 128:(i + 1) * 128, 0:seq // 2])
                    hd1 = nc.scalar.dma_start(
                        out=h[:, seq // 2:seq],
                        in_=hsT.ap()[i * 128:(i + 1) * 128, seq // 2:seq])
                    h_dmas.append(hd1 if i % 2 else hd0)
                    for s in range(NS):
                        nc.tensor.matmul(psk[s][:], kvw_sb[:, i, 0:128],
                                         h[:, s * 512:(s + 1) * 512],
                                         start=(i == 0), stop=(i == IC - 1))
                        nc.tensor.matmul(psv[s][:], kvw_sb[:, i, 128:256],
                                         h[:, s * 512:(s + 1) * 512],
                                         start=(i == 0), stop=(i == IC - 1))
                # hsq prefetch rides the gpsimd (SWDGE) queue in four slices,
                # gated behind the kv stream so it doesn't steal its bandwidth
                hsq_ap = hsq.ap().rearrange("(i p) q -> p i q", p=128)
                for ci in range(4):
                    hsq_dma = nc.gpsimd.dma_start(
                        out=hsq_sb[:, ci * 4:(ci + 1) * 4, :],
                        in_=hsq_ap[:, ci * 4:(ci + 1) * 4, :])
                    add_dep_helper(hsq_dma.ins, h_dmas[10 + ci].ins,
                                   reason="pace hsq behind the kv stream")
                for s in range(NS):
                    nc.scalar.activation(kT[:, s * 512:(s + 1) * 512], psk[s][:],
                                         mybir.ActivationFunctionType.Identity,
                                         bias=kvb_sb[:, 0:1])
                    nc.vector.tensor_scalar_add(vT[:, s * 512:(s + 1) * 512], psv[s][:],
                                                kvb_sb[:, 1:2])
            with tc.tile_pool(name="pst", bufs=4, space="PSUM") as pst:
                for t in range(T):
                    pt = pst.tile([128, 128], A_DT, tag="pt")
                    nc.tensor.transpose(pt[:], vT[:, t * 128:(t + 1) * 128], ident[:])
                    if t % 2 == 0:
                        nc.vector.tensor_copy(v[:, t, :], pt[:])
                    else:
                        nc.scalar.activation(v[:, t, :], pt[:],
                                             mybir.ActivationFunctionType.Copy)

        # ---------------- phase A: q-proj + scores -> exp -> PV, per head-pair ----
        ow_sb = owp.tile([128, nh, dmodel], O_DT)
        qw_cm = tc.tile_pool(name="qwp", bufs=2)
        qwp = qw_cm.__enter__()
        dn_cm = tc.tile_pool(name="dnp", bufs=2)
        dnp = dn_cm.__enter__()
        rs_cm = tc.tile_pool(name="rsp", bufs=2)
        rsp = rs_cm.__enter__()
        pexp_cm = tc.tile_pool(name="pexp", bufs=6)
        pexp = pexp_cm.__enter__()
        psS_cm = tc.tile_pool(name="psS", bufs=3, space="PSUM")
        psSp = psS_cm.__enter__()
        psU_cm = tc.tile_pool(name="psU", bufs=1, space="PSUM")
        psUp = psU_cm.__enter__()

        kv_gate = h_dmas  # kv-stream DMAs; pace phase-A prefetches behind them

        tail = {}   # deferred tail work of the previous head-pair

        def run_tail():
            if not tail:
                return
            heads, dn, psu = tail["heads"], tail["dn"], tail["psu"]
            # evict unnormalized PV accumulators
            for hh, hd in enumerate(heads):
                nc.vector.tensor_copy(attnT[:, hd, :], psu[hh][:])
            # denominator: partition-reduce dn via M=1 ones matmuls.  pd reuses
            # the freshly-evicted psu0 bank, the broadcast pr reuses psu1, so
            # the scores ("ss") rotation never couples to this tail work.
            pd_of = {0: psUp.tile([128, NQ], F32, tag="psu0", name="pd0"),
                     1: psUp.tile([128, NQ], F32, tag="psu1", name="pd1")}
            for hh in range(HG):
                nc.tensor.matmul(pd_of[hh][0:1, :], ones_a[:, 0:1],
                                 dn[:, hh, :], start=True, stop=True,
                                 skip_group_check=True)
            rs = rsp.tile([128, HG, NQ], F32, tag="rs", name="rs")
            rsh = rsp.tile([128, HG, NQ], A_DT, tag="rsh", name="rsh")
            for hh in range(HG):
                nc.vector.reciprocal_approx_fast(
                    rs[0:1, hh, :], pd_of[hh][0:1, :])
                with nc.allow_low_precision(reason="1/denom in fp16 is plenty"):
                    nc.vector.tensor_copy(rsh[0:1, hh, :], rs[0:1, hh, :])
            pbr = {0: psUp.tile([128, NQ], F32, tag="psu0", name="pbr0"),
                   1: psUp.tile([128, NQ], F32, tag="psu1", name="pbr1")}
            for hh, hd in enumerate(heads):
                nc.tensor.matmul(pbr[hh][:], ones_a[0:1, 0:128],
                                 rsh[0:1, hh, :],
                                 start=True, stop=True, skip_group_check=True)
                nc.vector.tensor_mul(attnT[:, hd, :], attnT[:, hd, :], pbr[hh][:])
            tail.clear()

        for hg in range(NHG):
            heads = [hg * HG + i for i in range(HG)]
            # stream this pair's weights: qw chunk (sync q), ow rows (scalar q)
            qwg = qwp.tile([128, IC, HG * 128], Q_DT, tag="qwg", name="qwg")
            qwg_ap = (qwT.ap()[:, hg * HG * 128:(hg + 1) * HG * 128]
                      .rearrange("(i p) o -> p i o", p=128))
            for ci in range(2):
                qwg_dma = nc.gpsimd.dma_start(
                    out=qwg[:, ci * 8:(ci + 1) * 8, :],
                    in_=qwg_ap[:, ci * 8:(ci + 1) * 8, :])
                if hg < 2:
                    add_dep_helper(qwg_dma.ins, kv_gate[12 + 2 * hg + ci].ins,
                                   reason="pace qw prefetch behind the kv stream")
            # q-projection for the pair (uses the "ss" PSUM region)
            psq = psSp.tile([128, HG, 512], F32, tag="ss", name="psq")
            last_psq = None
            for hh in range(HG):
                for i in range(IC):
                    last_psq = nc.tensor.matmul(
                        psq[:, hh, :], qwg[:, i, hh * 128:(hh + 1) * 128],
                        hsq_sb[:, i, :],
                        start=(i == 0), stop=(i == IC - 1),
                        skip_group_check=True)
            for hd in heads:
                ow_dma = nc.sync.dma_start(out=ow_sb[:, hd, :],
                                           in_=owT.ap()[hd * 128:(hd + 1) * 128, :])
                add_dep_helper(ow_dma.ins, last_psq.ins,
                               reason="pace ow stream behind phase-A progress")
            for hh, hd in enumerate(heads):
                nc.vector.tensor_scalar_add(qT[:, hd, :], psq[:, hh, :],
                                            qb_sb[:, hd:hd + 1])
            # deferred tail of the previous pair (keeps PE fed during our exps)
            run_tail()

            dn = dnp.tile([128, HG, NQ], A_DT, tag="dn", name="dn")
            nc.gpsimd.memset(dn[:], 0.0)
            psu = [psUp.tile([128, NQ], F32, tag=f"psu{hh}", name=f"psu{hh}")
                   for hh in range(HG)]
            pipe = []   # software-pipelined PV: run PV(kb-2) after scores(kb)
            for kb in range(T):
                j0 = j0_of(kb)
                ncols = (NSLOT - j0) * 128
                m = kb % 4
                ss = psSp.tile([128, HG, 512], F32, tag="ss", name="ss")
                for hh, hd in enumerate(heads):
                    nc.tensor.matmul(ss[:, hh, :ncols], kT[:, kb * 128:(kb + 1) * 128],
                                     qT[:, hd, j0 * 128:NQ], start=True, stop=False,
                                     skip_group_check=True)
                for hh in range(HG):
                    # boundary-block causal mask: ss[:, hh, 0:128] += -30000*(~allowed)
                    nc.tensor.matmul(ss[:, hh, 0:128], ident[:], bm_sb[:, m],
                                     start=False, stop=True, skip_group_check=True)
                p = pexp.tile([128, HG, 512], A_DT, tag="p", name="p")
                nc.scalar.activation(p[:, :, :ncols], ss[:, :, :ncols],
                                     mybir.ActivationFunctionType.Exp,
                                     bias=lm_sb[:, kb:kb + 1], scale=SCALE)
                nc.vector.tensor_add(dn[:, :, j0 * 128:NQ], dn[:, :, j0 * 128:NQ],
                                     p[:, :, :ncols])
                pipe.append((kb, j0, ncols, p))
                if len(pipe) > 2:
                    pkb, pj0, pnc, pp = pipe.pop(0)
                    for hh in range(HG):
                        nc.tensor.matmul(psu[hh][:, pj0 * 128:NQ], v[:, pkb, :],
                                         pp[:, hh, :pnc],
                                         start=(pkb == 0), stop=False,
                                         skip_group_check=True)
            for pkb, pj0, pnc, pp in pipe:
                for hh in range(HG):
                    nc.tensor.matmul(psu[hh][:, pj0 * 128:NQ], v[:, pkb, :],
                                     pp[:, hh, :pnc],
                                     start=(pkb == 0), stop=(pkb == T - 1),
                                     skip_group_check=True)
            tail.update({"heads": heads, "dn": dn, "psu": psu})
        run_tail()

        psU_cm.__exit__(None, None, None)
        psS_cm.__exit__(None, None, None)
        pexp_cm.__exit__(None, None, None)
        rs_cm.__exit__(None, None, None)
        dn_cm.__exit__(None, None, None)
        qw_cm.__exit__(None, None, None)
        aio_cm.__exit__(None, None, None)

        # ---------------- phase O: out = attnT.T @ owT + o_b ----------------
        with tc.tile_pool(name="psO", bufs=2, space="PSUM") as psOp, \
             tc.tile_pool(name="ost", bufs=2) as ostp:
            for sp in range(NSLOT):
                pso = {dt: psOp.tile([128, 512], F32, tag=f"pso{dt}",
                                     name=f"pso{dt}")
                       for dt in range(ND)}
                for ih in range(nh):
                    for dt in range(ND):
                        nc.tensor.matmul(pso[dt][:],
                                         attnT[:, ih, sp * 128:(sp + 1) * 128],
                                         ow_sb[:, ih, dt * 512:(dt + 1) * 512],
                                         start=(ih == 0), stop=False,
                                         skip_group_check=True)
                for dt in range(ND):
                    nc.tensor.matmul(pso[dt][:], ones_row_o[:],
                                     ob_sb[:, dt * 512:(dt + 1) * 512],
                                     start=False, stop=True, skip_group_check=True)
                for dt in range(ND):
                    og = ostp.tile([128, 512], O_DT, tag=f"og{dt % 2}", name="og")
                    nc.vector.tensor_copy(og[:], pso[dt][:])
                    eng = nc.sync if dt % 2 == 0 else nc.scalar
                    eng.dma_start(
                        out=out.ap()[sp * 128:(sp + 1) * 128,
                                     dt * 512:(dt + 1) * 512],
                        in_=og[:])
        ow_cm.__exit__(None, None, None)

    nc.compile()
    return nc


def make_in_maps(hidden_states, sequence_mask, q_w, q_b, kv_w, kv_b, o_w, o_b,
                 seq, dmodel, nh):
    """Host-side shard prep -> list of 8 per-core input dicts."""
    T = seq // 128
    NSLOT = T // 4
    npq, npkv, npa, npo = _NP_OF[Q_DT], _NP_OF[KV_DT], _NP_OF[A_DT], _NP_OF[O_DT]
    f32 = np.float32

    qwT = np.ascontiguousarray(q_w.astype(f32).T).astype(npq)
    kvwT = np.ascontiguousarray(kv_w.astype(f32).T).astype(npkv)
    owT = np.ascontiguousarray(o_w.astype(f32).T).astype(npo)
    qb2 = np.ascontiguousarray(q_b.astype(f32).reshape(nh, 128).T)
    kvb2 = np.ascontiguousarray(kv_b.astype(f32).reshape(2, 128).T)
    ob2 = o_b.astype(f32).reshape(1, dmodel).astype(npo)
    ident = np.eye(128, dtype=npa)
    ones128 = np.ones((128, 128), dtype=f32)
    ones_o = np.ones((1, 128), dtype=npo)
    tri = (np.arange(128)[None, :] >= np.arange(128)[:, None]).astype(f32)  # [k,q] q>=k

    in_maps = []
    for c in range(NCORE):
        b, r = divmod(c, 4)
        qtiles = [r + 4 * j for j in range(NSLOT)]
        hsT = np.ascontiguousarray(hidden_states[:, b, :].astype(f32).T)
        hsq = np.ascontiguousarray(
            np.concatenate([hsT[:, t * 128:(t + 1) * 128] for t in qtiles], axis=1))
        lm = np.where(sequence_mask[b].astype(np.int64) != 0, 0.0, NEG).astype(f32)
        lm = np.ascontiguousarray(lm.reshape(T, 128).T)
        bm = np.empty((128, 4 * 128), dtype=npa)
        for m in range(4):
            blk = (np.ones((128, 128), f32) if m < r else
                   (tri if m == r else np.zeros((128, 128), f32)))
            bm[:, m * 128:(m + 1) * 128] = (NEG * (1.0 - blk)).astype(npa)
        in_maps.append({
            "hsT": hsT.astype(npkv), "hsq": hsq.astype(npq), "qwT": qwT,
            "kvwT": kvwT, "owT": owT, "qb": qb2, "kvb": kvb2, "obias": ob2,
            "logmask": lm, "bmask": bm, "ident": ident, "ones_in": ones128,
            "ones_o": ones_o,
        })
    return in_maps


def assemble(results, seq, dmodel, nh):
    T = seq // 128
    NSLOT = T // 4
    full = np.empty((seq, BATCH, dmodel), np.float32)
    for c in range(NCORE):
        b, r = divmod(c, 4)
        o = results[c]["out"].astype(np.float32)
        for j in range(NSLOT):
            t = r + 4 * j
            full[t * 128:(t + 1) * 128, b, :] = o[j * 128:(j + 1) * 128, :]
    return full


_CACHE = {}


def kernel(hidden_states, sequence_mask, q_w, q_b, kv_w, kv_b, o_w, o_b):
    hidden_states = np.asarray(hidden_states)
    sequence_mask = np.asarray(sequence_mask)
    key = (SEQ, DMODEL, NH)
    if key not in _CACHE:
        _CACHE[key] = _build(SEQ, DMODEL, NH)
    nc = _CACHE[key]
    in_maps = make_in_maps(hidden_states, sequence_mask,
                           np.asarray(q_w), np.asarray(q_b), np.asarray(kv_w),
                           np.asarray(kv_b), np.asarray(o_w), np.asarray(o_b),
                           SEQ, DMODEL, NH)
    res = run_bass_kernel_spmd(nc, in_maps, core_ids=list(range(NCORE)))
    return assemble(res.results, SEQ, DMODEL, NH)
